# revision 1
# baseline (speedup 1.0000x reference)
"""CosineRouter (moe_routing) Trainium2 Bass kernel.

kernel(h, prototypes) -> (mask_full, probs, logits_clean, logits_sel)
  h:          [16384, 4096] f32
  prototypes: [64, 2, 4096] f32
Outputs match the reference:
  mask_full   [16384, 64] bool   top-8 experts per token
  probs       [16384, 64] f32    masked+renormalized softmax
  logits_clean[16384, 64] f32    10 * logsumexp_P(cos sims)
  logits_sel  [16384, 64] f32    == logits_clean (router_temp = 1)

Distribution: data-parallel over tokens — h is split into 8 shards of
2048 tokens (one per NeuronCore); the small prototype table is
replicated. No cross-core communication.

Per-core pipeline (all fp32; exact to ~1e-6 of the fp32 reference):
  A. prototypes: square+accum (ACT) -> inv-norm (Newton-refined),
     PE-transpose to pT [d, ep]; the prototype inv-norm is folded into
     the simsT PSUM->SBUF copy as a per-partition scale.
  B. per 512-token group (postproc software-pipelined one group behind):
     - 4 DMAs of h tiles [128, 4096] (natural layout)
     - per d-chunk (32): 4 PE transposes h[t,d]->hT[d,t] into PSUM,
       PSUM->SBUF copy (DVE/ACT), one fp32 matmul accumulating
       simsT[ep=128, t=512] over the 32 chunks
     - token sum-of-squares (ACT square+accum) -> inv-norm r[t],
       folded into the postproc exp as a per-partition scale
     - postproc: simsT->SBUF (x inv_p), 4 PE re-transposes to
       sims[t, ep], exp(r*sims), pair-sum over P=2, Ln, x10 -> logits;
       top-8 via the DVE max8 instruction; mask = logits >= 8th max;
       probs = mask*exp(l - max) / (sum_masked + 1e-9 * sum_all).
All ACT functions are kept inside the single natural_log_exp_and_others
table set (sqrt is seeded as exp(0.5*ln) and Newton-refined) so only one
activation-table load is emitted.
"""
import functools
import hashlib
import os
import shutil

import numpy as np

import concourse.bass as bass
import concourse.mybir as mybir
import concourse.tile as tile
from concourse import bacc
from concourse import bass_utils
from concourse.masks import make_identity
import concourse.bacc as _bacc_mod

f32 = mybir.dt.float32
u8 = mybir.dt.uint8
AF = mybir.ActivationFunctionType
ALU = mybir.AluOpType
ts = bass.ts

N_CORES = 8
T_FULL = 16384
T_CORE = T_FULL // N_CORES   # 2048
D = 4096
EP = 128
E = 64
DCH = D // 128               # 32 d-chunks
TG = 512                     # tokens per group
TPG = TG // 128              # token tiles per group
SCALE = 10.0
EPS = 1e-6

_orig_get_act_tables = _bacc_mod.get_activation_tables


def _patch_act_tables():
    """Resolve every ACT function we use to the single
    natural_log_exp_and_others set so one table load covers the kernel.
    Host-side patch of the set->funcs map used by the load-placement
    pass; set ids keep their act_info.json order."""
    KEEP = "natural_log_exp_and_others"
    STRIP = {AF.Exp, AF.Ln, AF.Square, AF.Copy, AF.Identity}

    @functools.cache
    def patched_fn(module_arch):
        tables = _orig_get_act_tables(module_arch)
        return {name: (set(funcs) if name == KEEP else set(funcs) - STRIP)
                for name, funcs in tables.items()}

    _bacc_mod.get_activation_tables = patched_fn


def _install_neff_cache(cache_dir="/tmp/neff_cache"):
    """Disk-cache walrus NEFF compiles keyed by bir.json hash (the same
    kernel is recompiled on every fresh process otherwise)."""
    from concourse import bass2jax
    if getattr(bass2jax, "_router_neff_cache", False):
        return
    bass2jax._router_neff_cache = True
    os.makedirs(cache_dir, exist_ok=True)
    orig = bass2jax.compile_bir_kernel

    def cached(bir_json, tmpdir, neff_name="file.neff"):
        key = hashlib.sha256(
            bir_json if isinstance(bir_json, bytes) else bir_json.encode()
        ).hexdigest()[:24]
        hit = os.path.join(cache_dir, f"{key}.neff")
        if os.path.exists(hit):
            dst = os.path.join(tmpdir, neff_name)
            shutil.copy(hit, dst)
            return dst
        path = orig(bir_json, tmpdir, neff_name)
        try:
            shutil.copy(path, hit)
        except OSError:
            pass
        return path

    bass2jax.compile_bir_kernel = cached


def _inv_norm(nc, pool, out, ss, w, tag):
    """out = 1/(sqrt(ss) + 1e-6); sqrt seeded as exp(0.5*ln(ss)) and
    corrected with 2 Newton steps (+ exact DVE reciprocal)."""
    y = pool.tile([128, w], f32, tag=f"{tag}_y")
    lns = pool.tile([128, w], f32, tag=f"{tag}_ln")
    nc.scalar.activation(lns[:], ss[:], AF.Ln)
    nc.scalar.activation(y[:], lns[:], AF.Exp, scale=0.5)
    r = pool.tile([128, w], f32, tag=f"{tag}_r")
    t = pool.tile([128, w], f32, tag=f"{tag}_t")
    y2 = pool.tile([128, w], f32, tag=f"{tag}_y2")
    for i in range(2):
        src = y if i == 0 else y2
        dst = y2 if i == 0 else y
        nc.vector.reciprocal(r[:], src[:])
        nc.vector.tensor_mul(t[:], ss[:], r[:])        # ss / y
        nc.vector.tensor_add(t[:], src[:], t[:])       # y + ss/y
        nc.vector.tensor_scalar_mul(dst[:], t[:], 0.5)
    nc.vector.tensor_scalar_add(y[:], y[:], EPS)
    nc.vector.reciprocal(out[:], y[:])


def build_kernel(repeat: int = 1, n_groups: int = T_CORE // TG):
    _patch_act_tables()
    G = n_groups
    T = G * TG
    nc = bacc.Bacc("TRN2", target_bir_lowering=False, debug=False)

    h_d = nc.dram_tensor("h", [T, D], f32, kind="ExternalInput").ap()
    p_d = nc.dram_tensor("protos", [EP, D], f32, kind="ExternalInput").ap()
    o_logits = nc.dram_tensor("logits", [T, E], f32,
                              kind="ExternalOutput").ap()
    o_probs = nc.dram_tensor("probs", [T, E], f32, kind="ExternalOutput").ap()
    o_mask = nc.dram_tensor("mask", [T, E], u8, kind="ExternalOutput").ap()

    QD = D // 4   # 1024: a quarter of the depth (8 d-chunks)
    with tile.TileContext(nc) as tc:
        with tc.tile_pool(name="const", bufs=1) as cpool, \
             tc.tile_pool(name="pT", bufs=1) as pT_pool:
            ident = cpool.tile([128, 128], f32)
            make_identity(nc, ident[:])
            # pT as 4 quarter tiles: Tile deps are tile-granular, so
            # matmuls only wait for the quarter they actually read
            pT_q = []
            for q in range(4):
                pTq_t = pT_pool.tile([128, QD], f32, tag=f"pT{q}")
                pT_q.append(pTq_t)
            inv_p = pT_pool.tile([128, 1], f32, tag="invp")

            def pT_slice(ch):
                return pT_q[ch // 8][:, ts(ch % 8, 128)]

            # ---------- Phase A: prototypes ----------
            with tc.tile_pool(name="pA", bufs=1) as pA, \
                 tc.tile_pool(name="pAps", bufs=2, space="PSUM") as pAps:
                p_nq = []
                for q in range(4):
                    pnq_t = pA.tile([128, QD], f32, tag=f"pn{q}")
                    p_nq.append(pnq_t)
                for q in range(4):
                    nc.sync.dma_start(p_nq[q][:], p_d[:, ts(q, QD)])
                sq_scr = pA.tile([128, QD], f32)
                ss_p4 = pA.tile([128, 4], f32)
                for q in range(4):
                    nc.scalar.activation(sq_scr[:], p_nq[q][:], AF.Square,
                                         accum_out=ss_p4[:, q:q + 1])
                ss_pa = pA.tile([128, 2], f32)
                nc.vector.tensor_add(ss_pa[:], ss_p4[:, 0::2], ss_p4[:, 1::2])
                ss_p = pA.tile([128, 1], f32)
                nc.vector.tensor_add(ss_p[:], ss_pa[:, 0:1], ss_pa[:, 1:2])
                _inv_norm(nc, pA, inv_p, ss_p, 1, "pn")
                for ch in range(DCH):
                    tp = pAps.tile([128, 128], f32, tag="ptr")
                    nc.tensor.transpose(
                        tp[:], p_nq[ch // 8][:, ts(ch % 8, 128)], ident[:])
                    nc.vector.tensor_copy(pT_slice(ch), tp[:])

            # ---------- Phase B: token groups (pipelined postproc) -----
            with tc.tile_pool(name="hbuf", bufs=2 * TPG) as hpool, \
                 tc.tile_pool(name="hT", bufs=6) as hTpool, \
                 tc.tile_pool(name="work", bufs=2) as wpool, \
                 tc.tile_pool(name="small", bufs=2) as spool, \
                 tc.tile_pool(name="outb", bufs=2) as opool, \
                 tc.tile_pool(name="trps", bufs=4, space="PSUM") as trps, \
                 tc.tile_pool(name="accps", bufs=2, space="PSUM") as accps, \
                 tc.tile_pool(name="strps", bufs=2, space="PSUM") as strps:

                state = {}

                def main(g, rep):
                    t0 = g * TG
                    h_tiles = []
                    for i in range(TPG):
                        ht = hpool.tile([128, D], f32, tag="h")
                        nc.sync.dma_start(
                            ht[:], h_d[t0 + i * 128: t0 + (i + 1) * 128, :])
                        h_tiles.append(ht)

                    acc = accps.tile([128, TG], f32, tag="acc")
                    if g == 0 and rep == 0:
                        # warm-up group: per-token-tile chunk streams so the
                        # PE starts right after the first h DMA lands
                        for i in range(TPG):
                            for ch in range(DCH):
                                hT_ps = trps.tile([128, TG], f32, tag="tr")
                                nc.tensor.transpose(
                                    hT_ps[:, 0:128],
                                    h_tiles[i][:, ts(ch, 128)], ident[:])
                                hT_sb = hTpool.tile([128, TG], f32, tag="hT")
                                if ch % 3 == 0:
                                    nc.scalar.copy(hT_sb[:, 0:128],
                                                   hT_ps[:, 0:128])
                                else:
                                    nc.vector.tensor_copy(hT_sb[:, 0:128],
                                                          hT_ps[:, 0:128])
                                nc.tensor.matmul(
                                    acc[:, ts(i, 128)], pT_slice(ch),
                                    hT_sb[:, 0:128], start=(ch == 0),
                                    stop=(ch == DCH - 1))
                    else:
                        for ch in range(DCH):
                            hT_ps = trps.tile([128, TG], f32, tag="tr")
                            for i in range(TPG):
                                nc.tensor.transpose(
                                    hT_ps[:, ts(i, 128)],
                                    h_tiles[i][:, ts(ch, 128)], ident[:])
                            hT_sb = hTpool.tile([128, TG], f32, tag="hT")
                            if ch % 3 == 0:
                                nc.scalar.copy(hT_sb[:], hT_ps[:])
                            else:
                                nc.vector.tensor_copy(hT_sb[:], hT_ps[:])
                            nc.tensor.matmul(acc[:], pT_slice(ch),
                                             hT_sb[:], start=(ch == 0),
                                             stop=(ch == DCH - 1))

                    ss_g = spool.tile([128, TPG * 2], f32, tag="ss")
                    sq_scr = wpool.tile([128, 2048], f32, tag="sqs")
                    for i in range(TPG):
                        for half in range(2):
                            nc.scalar.activation(
                                sq_scr[:],
                                h_tiles[i][:, half * 2048:(half + 1) * 2048],
                                AF.Square,
                                accum_out=ss_g[:, i * 2 + half:
                                               i * 2 + half + 1])
                    ss2 = spool.tile([128, TPG], f32, tag="ss2")
                    nc.vector.tensor_add(ss2[:], ss_g[:, 0::2], ss_g[:, 1::2])
                    inv_g = spool.tile([128, TPG], f32, tag="invg")
                    _inv_norm(nc, spool, inv_g, ss2, TPG, "hn")
                    state[g] = (acc, inv_g)

                def post(g, rep):
                    t0 = g * TG
                    acc, inv_g = state.pop(g)
                    simsT = wpool.tile([128, TG], f32, tag="simsT")
                    # prototype inv-norm folded in (per-partition = per-ep)
                    nc.vector.tensor_scalar_mul(simsT[:], acc[:], inv_p[:])
                    logits_g = opool.tile([128, TPG * E], f32, tag="lg")
                    probs_g = opool.tile([128, TPG * E], f32, tag="pg")
                    mask_g = opool.tile([128, TPG * E], u8, tag="mg")

                    for i in range(TPG):
                        s_ps = strps.tile([128, 128], f32, tag="str")
                        nc.tensor.transpose(s_ps[:], simsT[:, ts(i, 128)],
                                            ident[:])
                        r_i = inv_g[:, i:i + 1]
                        E2 = spool.tile([128, 128], f32, tag="E2")
                        # token inv-norm folded into the exp
                        nc.scalar.activation(E2[:], s_ps[:], AF.Exp,
                                             scale=r_i)
                        S2 = spool.tile([128, E], f32, tag="S2")
                        nc.vector.tensor_add(S2[:], E2[:, 0::2], E2[:, 1::2])
                        L = spool.tile([128, E], f32, tag="L")
                        nc.scalar.activation(L[:], S2[:], AF.Ln)
                        lg = logits_g[:, ts(i, E)]
                        nc.vector.tensor_scalar_mul(lg, L[:], SCALE)
                        mx = spool.tile([128, 8], f32, tag="mx")
                        nc.vector.max(out=mx[:], in_=lg)
                        mask_f = spool.tile([128, E], f32, tag="mf")
                        nc.vector.tensor_scalar(mask_f[:], lg, mx[:, 7:8],
                                                None, op0=ALU.is_ge)
                        nc.vector.tensor_copy(mask_g[:, ts(i, E)], mask_f[:])
                        negm = spool.tile([128, 1], f32, tag="negm")
                        nc.vector.tensor_scalar_mul(negm[:], mx[:, 0:1], -1.0)
                        Ex = spool.tile([128, E], f32, tag="Ex")
                        nc.scalar.activation(Ex[:], lg, AF.Exp,
                                             bias=negm[:], scale=1.0)
                        Z = spool.tile([128, 1], f32, tag="Z")
                        nc.vector.reduce_sum(Z[:], Ex[:],
                                             axis=mybir.AxisListType.X)
                        mE = spool.tile([128, E], f32, tag="mE")
                        S8 = spool.tile([128, 1], f32, tag="S8")
                        nc.vector.tensor_mul(mE[:], Ex[:], mask_f[:])
                        nc.vector.reduce_sum(S8[:], mE[:],
                                             axis=mybir.AxisListType.X)
                        den = spool.tile([128, 1], f32, tag="den")
                        nc.vector.tensor_scalar(den[:], Z[:], 1e-9,
                                                S8[:, 0:1],
                                                op0=ALU.mult, op1=ALU.add)
                        rcp = spool.tile([128, 1], f32, tag="rcp")
                        nc.vector.reciprocal(rcp[:], den[:])
                        nc.vector.tensor_scalar_mul(probs_g[:, ts(i, E)],
                                                    mE[:], rcp[:])

                    if rep == repeat - 1:
                        dv_l = o_logits[t0:t0 + TG, :].rearrange(
                            "(i p) e -> p i e", p=128)
                        dv_p = o_probs[t0:t0 + TG, :].rearrange(
                            "(i p) e -> p i e", p=128)
                        dv_m = o_mask[t0:t0 + TG, :].rearrange(
                            "(i p) e -> p i e", p=128)
                        nc.sync.dma_start(dv_l, logits_g[:].rearrange(
                            "p (i e) -> p i e", i=TPG))
                        nc.sync.dma_start(dv_p, probs_g[:].rearrange(
                            "p (i e) -> p i e", i=TPG))
                        nc.sync.dma_start(dv_m, mask_g[:].rearrange(
                            "p (i e) -> p i e", i=TPG))

                prev = None
                for rep in range(repeat):
                    for g in range(G):
                        main(g, rep)
                        if prev is not None:
                            post(*prev)
                        prev = (g, rep)
                post(*prev)

    nc.compile()
    return nc


_nc_cache = {}


def _get_nc(repeat=1):
    if repeat not in _nc_cache:
        _install_neff_cache()
        _nc_cache[repeat] = build_kernel(repeat=repeat)
    return _nc_cache[repeat]


def kernel(h: np.ndarray, prototypes: np.ndarray):
    h = np.ascontiguousarray(np.asarray(h, dtype=np.float32))
    protos2d = np.ascontiguousarray(
        np.asarray(prototypes, dtype=np.float32).reshape(EP, D))
    assert h.shape == (T_FULL, D), h.shape

    nc = _get_nc()
    in_maps = [{"h": h[c * T_CORE:(c + 1) * T_CORE], "protos": protos2d}
               for c in range(N_CORES)]
    res = bass_utils.run_bass_kernel_spmd(
        nc, in_maps, core_ids=list(range(N_CORES)))

    logits = np.concatenate([r["logits"] for r in res.results], axis=0)
    probs = np.concatenate([r["probs"] for r in res.results], axis=0)
    mask = np.concatenate([r["mask"] for r in res.results],
                          axis=0).astype(bool)
    return mask, probs, logits, logits.copy()

